# revision 10
# baseline (speedup 1.0000x reference)
"""Trainium2 Bass kernel for nn_DNN_Model_33852932227151.

Per-sample pipeline (see reference):
  theta1 = MLP(sample1)            303 -> 1024 -> 1024 -> 512 -> 264
  F1, F2 normalized precoders      (cols 200:264)
  theta  = unit-modulus phases     (cols 0:200 as complex [100])
  CCC_bc = Re(theta^H T_bc theta) / 1e-15 ; scale = rsqrt(max(max_c CCC, 1))
  out    = [Re(theta*scale), Im(theta*scale), Re F1, Im F1, Re F2, Im F2]

Sharding: pure data parallel over batch: 2048 = 8 cores x 256 samples;
weights replicated.

Numerical constraint (measured on the true harness inputs): samples exist
with |max_c quad| as small as 1.3e-4 * sigma(quad); a branch flip there
puts theta_hat at O(1) vs O(1e-7) and fails the grade. The contraction
therefore runs fp32 end-to-end. The MLP runs plain fp32 on the PE
(measured: PE fp32 is only ~1.4x a bf16 matmul here, so fp32 beats
hi/lo bf16 splitting outright and is exact w.r.t. the reference MLP).

Quadratic form, packed by circular diagonals (host prep, free w.r.t. HW):
  Re(theta^H T theta)
    = sum_{k=0..50, n} Ts[k,n] * (a_n a_m + b_n b_m)
                     + Ta[k,n] * (b_n a_m - a_n b_m),   m = (n+k) % 100
  Ts[k,n] = Tr[n,m] + Tr[m,n] (k>0; diag at k=0), Ta[k,n] = Ti[n,m] - Ti[m,n]
  (k=50 half-row zeroed to avoid double count)
This halves HBM traffic (41 MB/core), DVE multiply work, AND the
outer-product build work vs the dense [100,100] formulation. On device the
m-index is realized with an overlapping-stride access pattern over
a_ext = [a | a] (stride 1 on both the k and n axes), so the packed outer
products are built with 6 plain DVE passes per group, no gathers.
Per chunk: DVE multiply, reduce on the scalar engine (ACT accumulate).
"""

import os
import threading

import ml_dtypes
import numpy as np

import concourse.bass as bass
from concourse import bacc
import concourse.mybir as mybir
import concourse.tile as tile
from concourse.bass_utils import run_bass_kernel_spmd

F32 = mybir.dt.float32
BF16 = mybir.dt.bfloat16
NP_BF16 = ml_dtypes.bfloat16

# ---- problem constants (hardcoded per harness contract) ----
B = 2048
N_CORES = 8
B_LOC = B // N_CORES          # 256 samples per core
DIN = 303
DIN_PAD = 384                 # 3 * 128
H1, H2, H3 = 1024, 1024, 512
DOUT = 264
NRIS = 100
C = 4
MN = 16
INV_THRESH = 1.0e15           # 1 / THRESH_W

N_GROUPS = B_LOC // 128       # 2 sample groups of 128 per core
NK = 51                       # circular diagonal offsets 0..50

TCH_BUFS = int(os.environ.get("KERNEL_TCH_BUFS", "3"))
# T chunk load path: "swdge" (gpsimd queues, 16 engines) | "hwdge" (sync)
T_DMA = os.environ.get("KERNEL_T_DMA", "hwdge")
# outer-product build engines: "dve" | "gpsimd" | "split"
BUILD = os.environ.get("KERNEL_BUILD", "split")


def build_nc():
    nc = bacc.Bacc(trn_type="TRN2", debug=False)

    # ---- DRAM parameters (per-core shard shapes, host-prepped layouts) ----
    x0d = nc.declare_dram_parameter("x0", [128, 3, B_LOC], F32, isOutput=False)
    w1d = nc.declare_dram_parameter("w1", [128, 3, H1], F32, isOutput=False)
    w2d = nc.declare_dram_parameter("w2", [128, 8, H2], F32, isOutput=False)
    w3d = nc.declare_dram_parameter("w3", [128, 8, H3], F32, isOutput=False)
    w4d = nc.declare_dram_parameter("w4", [128, 4, DOUT], F32, isOutput=False)
    biases = nc.declare_dram_parameter("biases", [128, 24], F32, isOutput=False)
    tpack = nc.declare_dram_parameter("tpack", [B_LOC, C, 2, NK, NRIS], F32,
                                      isOutput=False)
    out = nc.declare_dram_parameter("out", [B_LOC, DOUT], F32, isOutput=True)

    ident_dram = nc.inline_tensor(np.eye(128, dtype=np.float32), name="ident128")

    with tile.TileContext(nc) as tc:
        _emit(tc, x0d, w1d, w2d, w3d, w4d, biases, tpack, out, ident_dram)
    nc.compile()
    return nc


def _diag_view(ext, sh3):
    # ext [128, 200] -> AP [128, NK, 100] with elem (s, k, n) = ext[s, n + k]
    v = ext[:, None, 0:NRIS].to_broadcast(sh3).copy()
    t = v.ap
    t[1] = [1, NK]
    v.ap = t
    return v


def _emit(tc, x0d, w1d, w2d, w3d, w4d, biases_d, tpack, out, ident_dram):
    nc = tc.nc

    with (
        tc.tile_pool(name="consts", bufs=1) as consts,
        tc.tile_pool(name="acts", bufs=1) as acts,
        tc.tile_pool(name="theta", bufs=1) as theta_pool,
        tc.tile_pool(name="tch", bufs=TCH_BUFS) as tch_pool,
        tc.tile_pool(name="tsc", bufs=2) as tsc_pool,
        tc.tile_pool(name="psmm", bufs=4, space="PSUM") as psmm,
        tc.tile_pool(name="pstr", bufs=2, space="PSUM") as pstr,
    ):
        ident = consts.tile([128, 128], F32)
        nc.sync.dma_start(out=ident, in_=ident_dram[:, :])
        bs = consts.tile([128, 24], F32)
        nc.sync.dma_start(out=bs, in_=biases_d[:, :])

        # ---------------- MLP (feature-major, fp32) ----------------
        # PE fp32 here is ~1.4x a bf16 matmul per (k,m) pair, so plain fp32
        # beats hi/lo bf16 splitting (3-4 matmuls/pair) outright and is
        # exact w.r.t. the fp32 reference MLP.
        with tc.tile_pool(name="weights", bufs=1) as wpool:
            def stage(dram, shape, tag):
                t = wpool.tile(shape, F32, name=tag, tag=tag)
                nc.sync.dma_start(out=t, in_=dram[:, :, :])
                return t

            # critical path first: x0 + W1, then the rest streams behind
            x0 = stage(x0d, [128, 3, B_LOC], "x0")
            w1 = stage(w1d, [128, 3, H1], "w1")
            w2 = stage(w2d, [128, 8, H2], "w2")
            w3 = stage(w3d, [128, 8, H3], "w3")
            w4 = stage(w4d, [128, 4, DOUT], "w4")

            def dense(x, n_k, w, n_m, bias0, relu, out_t, m_widths=None):
                # out[feat, batch] = act(W.T @ x + b); K = n_k*128 partitions
                for mo in range(n_m):
                    if m_widths is None:
                        mw, m_lo = 128, mo * 128
                    else:
                        m_lo, mw = m_widths[mo]
                    ps = psmm.tile([128, B_LOC], F32, tag="mm")
                    for k in range(n_k):
                        nc.tensor.matmul(ps[0:mw, :],
                                         lhsT=w[:, k, m_lo:m_lo + mw],
                                         rhs=x[:, k, :],
                                         start=(k == 0), stop=(k == n_k - 1))
                    bcol = bs[0:mw, bias0 + mo:bias0 + mo + 1]
                    if relu:
                        nc.scalar.activation(
                            out=out_t[0:mw, mo, :], in_=ps[0:mw, :],
                            func=mybir.ActivationFunctionType.Relu,
                            bias=bcol, scale=1.0)
                    else:
                        nc.vector.tensor_scalar(
                            out=out_t[0:mw, mo, :], in0=ps[0:mw, :],
                            scalar1=bcol, scalar2=None,
                            op0=mybir.AluOpType.add)

            h1 = wpool.tile([128, 8, B_LOC], F32, name="h1", tag="h1")
            dense(x0, 3, w1, 8, 0, True, h1)
            h2 = wpool.tile([128, 8, B_LOC], F32, name="h2", tag="h2")
            dense(h1, 8, w2, 8, 8, True, h2)
            h3 = wpool.tile([128, 4, B_LOC], F32, name="h3", tag="h3")
            dense(h2, 8, w3, 4, 16, True, h3)
            # Layer 4 (no relu), fp32 out, aligned chunks:
            #   [0:100] Re(theta_raw), [100:200] Im(theta_raw), [200:264] F
            thp = acts.tile([128, 3, B_LOC], F32)
            dense(h3, 4, w4, 3, 20, False, thp,
                  m_widths=[(0, 100), (100, 100), (200, 64)])

        # ---------------- unit-modulus theta (feature-major) ----------------
        p_re = thp[0:100, 0, :]
        p_im = thp[0:100, 1, :]
        sq = tsc_pool.tile([100, B_LOC], F32, tag="sq")
        sq2 = tsc_pool.tile([100, B_LOC], F32, tag="sq2")
        nc.vector.tensor_mul(sq, p_re, p_re)
        nc.vector.tensor_mul(sq2, p_im, p_im)
        nc.vector.tensor_add(sq, sq, sq2)
        nc.scalar.sqrt(sq, sq)
        nc.vector.reciprocal(sq, sq)               # sq = 1/|theta|
        # [128, *] tiles so PE transposes use a full K=128 contraction
        # (sub-128-K matmuls misbehave); rows 100:128 are garbage that the
        # post-transpose column slice discards.
        a_fm = theta_pool.tile([128, B_LOC], F32)  # Re(theta), unit modulus
        b_fm = theta_pool.tile([128, B_LOC], F32)  # Im(theta)
        nc.vector.tensor_mul(a_fm[0:100, :], p_re, sq)
        nc.vector.tensor_mul(b_fm[0:100, :], p_im, sq)

        # ---------------- per-group sample-major processing ----------------
        # obuild opens after the weights pool closed: its arena reuses the
        # freed weight space so peak SBUF stays under the Tile cap.
        obuild_cm = tc.tile_pool(name="obuild", bufs=2)
        obuild = obuild_cm.__enter__()
        obuild1_cm = tc.tile_pool(name="obuild1", bufs=1)
        obuild1 = obuild1_cm.__enter__()
        ccc_all = theta_pool.tile([128, N_GROUPS, C], F32)
        for g in range(N_GROUPS):
            gs = slice(g * 128, (g + 1) * 128)

            def to_sample_major(src_fm, np_, tag):
                # src [128, 128] feature-major slice -> [128, np_] sample-major
                ps = pstr.tile([128, 128], F32, tag="tr")
                nc.tensor.transpose(ps, src_fm, ident)
                dst = theta_pool.tile([128, np_], F32, name=tag, tag=tag)
                nc.scalar.copy(out=dst, in_=ps[:, 0:np_])
                return dst

            a_pack = to_sample_major(a_fm[:, gs], 100, f"apack{g}")
            b_pack = to_sample_major(b_fm[:, gs], 100, f"bpack{g}")
            f_pack = to_sample_major(thp[:, 2, gs], 64, f"fpack{g}")

            # ---- F1/F2 precoder normalization (sample-major) ----
            fsq = tsc_pool.tile([128, 2, 32], F32, tag="fsq")
            f_v = f_pack[:].rearrange("p (g2 i) -> p g2 i", g2=2)
            nc.vector.tensor_mul(fsq, f_v, f_v)
            fnorm = tsc_pool.tile([128, 2], F32, tag="fnorm")
            nc.vector.reduce_sum(fnorm, fsq, axis=mybir.AxisListType.X)
            # scale = sqrt(2/norm) = 1/sqrt(norm*0.5)
            nc.scalar.activation(out=fnorm, in_=fnorm,
                                 func=mybir.ActivationFunctionType.Sqrt, scale=0.5)
            nc.vector.reciprocal(fnorm, fnorm)
            fhat = theta_pool.tile([128, 2, 32], F32, name=f"fhat{g}", tag=f"fhat{g}")
            nc.vector.tensor_mul(fhat, f_v,
                                 fnorm[:, :, None].to_broadcast((128, 2, 32)))
            nc.sync.dma_start(out=out[gs, 200:264],
                              in_=fhat[:].rearrange("p g2 i -> p (g2 i)"))

            # ---- packed fp32 outer products over circular diagonals ----
            # ovec[:, 0, k, n] = a_n a_m + b_n b_m   (m = (n+k) % 100)
            # ovec[:, 1, k, n] = b_n a_m - a_n b_m
            # The shifted operand a_m = a_ext[n+k] is an overlapping-stride
            # view (runs ~3x slower than plain on the compute engines, but
            # only 3 of the 6 ops read it and they overlap the stream).
            # o1 builds first so side-0 chunks start consuming while the
            # GPSIMD chain finishes o2.
            a_ext = theta_pool.tile([128, 2 * NRIS], F32,
                                    name=f"aext{g}", tag=f"aext{g}")
            b_ext = theta_pool.tile([128, 2 * NRIS], F32,
                                    name=f"bext{g}", tag=f"bext{g}")
            nc.scalar.copy(out=a_ext[:, 0:NRIS], in_=a_pack)
            nc.scalar.copy(out=a_ext[:, NRIS:2 * NRIS], in_=a_pack)
            nc.scalar.copy(out=b_ext[:, 0:NRIS], in_=b_pack)
            nc.scalar.copy(out=b_ext[:, NRIS:2 * NRIS], in_=b_pack)
            sh3 = (128, NK, NRIS)
            a_sh = _diag_view(a_ext, sh3)                # a_m
            b_sh = _diag_view(b_ext, sh3)                # b_m
            a_b = a_pack[:, None, :].to_broadcast(sh3)   # a_n
            b_b = b_pack[:, None, :].to_broadcast(sh3)   # b_n
            ovec = obuild.tile([128, 2, NK, NRIS], F32, tag="ovec")
            # the only build scratch is the ACT-accumulate dump tile (free
            # during builds; the WAR edge only delays group g+1's ba/sub
            # ops to the end of group g's stream, where they hide)
            act_dump = obuild1.tile([128, NK, NRIS], F32, tag="dump")

            def build_op(eng, out_, in0, in1, op):
                if eng == "d":
                    nc.vector.tensor_tensor(out_, in0, in1, op)
                else:
                    nc.gpsimd.tensor_tensor(out_, in0, in1, op)
            mult = mybir.AluOpType.mult
            # engine split: DVE does o1 (aa, +bb); GPSIMD does bb and the
            # whole o2 chain concurrently
            e = {"dve": "dddddd", "gpsimd": "gggggg",
                 "split": "dgdggg"}[BUILD]
            build_op(e[0], ovec[:, 0], a_b, a_sh, mult)            # aa
            build_op(e[1], act_dump, b_b, b_sh, mult)              # bb
            build_op(e[2], ovec[:, 0], ovec[:, 0], act_dump,
                     mybir.AluOpType.add)                          # o1 = aa+bb
            build_op(e[3], ovec[:, 1], b_b, a_sh, mult)            # ba
            build_op(e[4], act_dump, a_b, b_sh, mult)              # ab
            build_op(e[5], ovec[:, 1], ovec[:, 1], act_dump,
                     mybir.AluOpType.subtract)                     # o2 = ba-ab

            # ---- stream packed T; per chunk: DVE multiply, ACT reduce ----
            # (tensor_tensor_reduce crashes this HW stack, so the reduce
            # rides scalar-engine activation(Copy, accum_out=...); the 1e15
            # CCC scale is applied later in fp32)
            parts = theta_pool.tile([128, C, 2], F32,
                                    name=f"parts{g}", tag=f"parts{g}")
            ccc = ccc_all[:, g, :]
            for side in range(2):
                for c in range(C):
                    chunk = tch_pool.tile([128, NK, NRIS], F32, tag="tchunk")
                    src = tpack[gs, c, side]
                    if T_DMA == "swdge":
                        nc.gpsimd.dma_start(out=chunk, in_=src)
                    else:
                        nc.sync.dma_start(out=chunk, in_=src)
                    nc.vector.tensor_mul(chunk, chunk, ovec[:, side])
                    nc.scalar.activation(
                        out=act_dump, in_=chunk,
                        func=mybir.ActivationFunctionType.Copy,
                        bias=0.0, scale=1.0,
                        accum_out=parts[:, c, side:side + 1])
            nc.vector.reduce_sum(ccc, parts, axis=mybir.AxisListType.X)

            # ---- scale = rsqrt(max(CCC*1e15, 1)) ; theta_hat ----
            mx = tsc_pool.tile([128, 1], F32, tag="mx")
            nc.vector.reduce_max(mx, ccc, axis=mybir.AxisListType.X)
            nc.vector.tensor_scalar(out=mx, in0=mx, scalar1=INV_THRESH,
                                    scalar2=1.0, op0=mybir.AluOpType.mult,
                                    op1=mybir.AluOpType.max)
            nc.scalar.sqrt(mx, mx)
            nc.vector.reciprocal(mx, mx)
            th_re = theta_pool.tile([128, NRIS], F32, name=f"thre{g}", tag=f"thre{g}")
            th_im = theta_pool.tile([128, NRIS], F32, name=f"thim{g}", tag=f"thim{g}")
            nc.vector.tensor_scalar_mul(th_re, a_pack, mx)
            nc.vector.tensor_scalar_mul(th_im, b_pack, mx)
            nc.sync.dma_start(out=out[gs, 0:100], in_=th_re)
            nc.sync.dma_start(out=out[gs, 100:200], in_=th_im)
        obuild1_cm.__exit__(None, None, None)
        obuild_cm.__exit__(None, None, None)


_NC_LOCK = threading.Lock()
_NC = None


def _get_nc():
    global _NC
    with _NC_LOCK:
        if _NC is None:
            _NC = build_nc()
    return _NC


def _wprep(W, n_k):
    # [K, M] fp32 -> lhsT layout [p, o, m] with k = o*128 + p
    K, M = W.shape
    Wp = np.zeros((n_k * 128, M), np.float32)
    Wp[:K] = W
    return np.ascontiguousarray(Wp.reshape(n_k, 128, M).transpose(1, 0, 2))


def _pack_T(t_re, t_im):
    """[B, C, 100, 100] fp32 pair -> [B, C, 2, NK, NRIS] circular-diagonal
    packed Ts/Ta (see module docstring)."""
    k_ix = np.arange(NK)[:, None]
    n_ix = np.arange(NRIS)[None, :]
    m_ix = (n_ix + k_ix) % NRIS                      # [NK, 100]
    n_bx = np.broadcast_to(n_ix, (NK, NRIS))
    ts = t_re[:, :, n_bx, m_ix] + t_re[:, :, m_ix, n_bx]
    ta = t_im[:, :, n_bx, m_ix] - t_im[:, :, m_ix, n_bx]
    ts[:, :, 0, :] = t_re.diagonal(axis1=2, axis2=3)  # k=0: plain diag
    ta[:, :, 0, :] = 0.0
    ts[:, :, 50, 50:] = 0.0                           # k=50: half, no double count
    ta[:, :, 50, 50:] = 0.0
    return np.ascontiguousarray(
        np.stack([ts, ta], axis=2).astype(np.float32))


def _prep_shared(inputs):
    """Host-side prep of replicated tensors (weights/biases) + packed T."""
    w1 = _wprep(np.asarray(inputs["W1"], np.float32), 3)
    w2 = _wprep(np.asarray(inputs["W2"], np.float32), 8)
    w3 = _wprep(np.asarray(inputs["W3"], np.float32), 8)
    w4 = _wprep(np.asarray(inputs["W4"], np.float32), 4)
    biases = np.zeros((128, 24), np.float32)
    biases[:, 0:8] = np.asarray(inputs["b1"], np.float32).reshape(8, 128).T
    biases[:, 8:16] = np.asarray(inputs["b2"], np.float32).reshape(8, 128).T
    biases[:, 16:20] = np.asarray(inputs["b3"], np.float32).reshape(4, 128).T
    b4 = np.asarray(inputs["b4"], np.float32)
    biases[0:100, 20] = b4[0:100]
    biases[0:100, 21] = b4[100:200]
    biases[0:64, 22] = b4[200:264]
    tpack = _pack_T(np.asarray(inputs["T_real"], np.float32),
                    np.asarray(inputs["T_imag"], np.float32))
    return w1, w2, w3, w4, biases, tpack


def _shard_inputs(inputs):
    w1, w2, w3, w4, biases, tpack = _prep_shared(inputs)
    s1 = np.asarray(inputs["sample1"], np.float32)
    in_maps = []
    for i in range(N_CORES):
        bsl = slice(i * B_LOC, (i + 1) * B_LOC)
        x = np.zeros((DIN_PAD, B_LOC), np.float32)
        x[:DIN] = s1[bsl].T
        x0 = np.ascontiguousarray(x.reshape(3, 128, B_LOC).transpose(1, 0, 2))
        in_maps.append({
            "x0": x0, "w1": w1, "w2": w2, "w3": w3, "w4": w4,
            "biases": biases,
            "tpack": tpack[bsl],
        })
    return in_maps


def run_on_hw(inputs, trace=False, **kwargs):
    nc = _get_nc()
    res = run_bass_kernel_spmd(nc, _shard_inputs(inputs),
                               list(range(N_CORES)), trace=trace, **kwargs)
    full = np.concatenate([res.results[i]["out"] for i in range(N_CORES)], axis=0)
    return full, res


def kernel(**inputs) -> np.ndarray:
    full, _ = run_on_hw(inputs, trace=False)
    return full.astype(np.float32)


# revision 15
# speedup vs baseline: 1.3633x; 1.3633x over previous
"""Trainium2 Bass kernel for nn_DNN_Model_33852932227151.

Per-sample pipeline (see reference):
  theta1 = MLP(sample1)            303 -> 1024 -> 1024 -> 512 -> 264
  F1, F2 normalized precoders      (cols 200:264)
  theta  = unit-modulus phases     (cols 0:200 as complex [100])
  CCC_bc = Re(theta^H T_bc theta) / 1e-15 ; scale = rsqrt(max(max_c CCC, 1))
  out    = [Re(theta*scale), Im(theta*scale), Re F1, Im F1, Re F2, Im F2]

Sharding: pure data parallel over batch: 2048 = 8 cores x 256 samples;
weights replicated.

Numerical constraint (measured on the true harness inputs): samples exist
with |max_c quad| as small as 1.3e-4 * sigma(quad); a branch flip there
puts theta_hat at O(1) vs O(1e-7) and fails the grade. The contraction
therefore runs fp32 end-to-end. The MLP runs plain fp32 on the PE
(measured: PE fp32 is only ~1.4x a bf16 matmul here, so fp32 beats
hi/lo bf16 splitting outright and is exact w.r.t. the reference MLP).

Quadratic form, packed by circular diagonals (host prep, free w.r.t. HW):
  Re(theta^H T theta)
    = sum_{k=0..50, n} Ts[k,n] * (a_n a_m + b_n b_m)
                     + Ta[k,n] * (b_n a_m - a_n b_m),   m = (n+k) % 100
  Ts[k,n] = Tr[n,m] + Tr[m,n] (k>0; diag at k=0), Ta[k,n] = Ti[n,m] - Ti[m,n]
  (k=50 half-row zeroed to avoid double count)
This halves HBM traffic (41 MB/core), DVE multiply work, AND the
outer-product build work vs the dense [100,100] formulation. On device the
m-index is realized with an overlapping-stride access pattern over
a_ext = [a | a] (stride 1 on both the k and n axes), so the packed outer
products are built with 6 plain DVE passes per group, no gathers.
Per chunk: DVE multiply, reduce on the scalar engine (ACT accumulate).
"""

import os
import threading

import ml_dtypes
import numpy as np

import concourse.bass as bass
from concourse import bacc
import concourse.mybir as mybir
import concourse.tile as tile
from concourse.bass_utils import run_bass_kernel_spmd

F32 = mybir.dt.float32
BF16 = mybir.dt.bfloat16
NP_BF16 = ml_dtypes.bfloat16

# ---- problem constants (hardcoded per harness contract) ----
B = 2048
N_CORES = 8
B_LOC = B // N_CORES          # 256 samples per core
DIN = 303
DIN_PAD = 384                 # 3 * 128
H1, H2, H3 = 1024, 1024, 512
DOUT = 264
NRIS = 100
C = 4
MN = 16
INV_THRESH = 1.0e15           # 1 / THRESH_W

N_GROUPS = B_LOC // 128       # 2 sample groups of 128 per core
NK = 51                       # circular diagonal offsets 0..50

TCH_BUFS = int(os.environ.get("KERNEL_TCH_BUFS", "3"))
# T chunk load path: "swdge" (gpsimd queues, 16 engines) | "hwdge" (sync)
T_DMA = os.environ.get("KERNEL_T_DMA", "hwdge")


def build_nc():
    nc = bacc.Bacc(trn_type="TRN2", debug=False)

    # ---- DRAM parameters (per-core shard shapes, host-prepped layouts) ----
    x0d = nc.declare_dram_parameter("x0", [128, 3, B_LOC], F32, isOutput=False)
    w1d = nc.declare_dram_parameter("w1", [128, 3, H1], F32, isOutput=False)
    w2d = nc.declare_dram_parameter("w2", [128, 8, H2], F32, isOutput=False)
    w3d = nc.declare_dram_parameter("w3", [128, 8, H3], F32, isOutput=False)
    w4d = nc.declare_dram_parameter("w4", [128, 4, DOUT], F32, isOutput=False)
    biases = nc.declare_dram_parameter("biases", [128, 24], F32, isOutput=False)
    tpack = nc.declare_dram_parameter("tpack", [B_LOC, C, 2, NK, NRIS], F32,
                                      isOutput=False)
    out = nc.declare_dram_parameter("out", [B_LOC, DOUT], F32, isOutput=True)

    ident_dram = nc.inline_tensor(np.eye(128, dtype=np.float32), name="ident128")

    with tile.TileContext(nc) as tc:
        _emit(tc, x0d, w1d, w2d, w3d, w4d, biases, tpack, out, ident_dram)
    nc.compile()
    return nc


def _diag_view(ext, sh3):
    # ext [128, 200] -> AP [128, NK, 100] with elem (s, k, n) = ext[s, n + k]
    v = ext[:, None, 0:NRIS].to_broadcast(sh3).copy()
    t = v.ap
    t[1] = [1, NK]
    v.ap = t
    return v


def _emit(tc, x0d, w1d, w2d, w3d, w4d, biases_d, tpack, out, ident_dram):
    nc = tc.nc

    with (
        tc.tile_pool(name="consts", bufs=1) as consts,
        tc.tile_pool(name="acts", bufs=1) as acts,
        tc.tile_pool(name="theta", bufs=1) as theta_pool,
        tc.tile_pool(name="tch", bufs=TCH_BUFS) as tch_pool,
        tc.tile_pool(name="tsc", bufs=2) as tsc_pool,
        tc.tile_pool(name="psmm", bufs=4, space="PSUM") as psmm,
        tc.tile_pool(name="pstr", bufs=2, space="PSUM") as pstr,
    ):
        ident = consts.tile([128, 128], F32)
        nc.sync.dma_start(out=ident, in_=ident_dram[:, :])
        bs = consts.tile([128, 24], F32)
        nc.sync.dma_start(out=bs, in_=biases_d[:, :])
        hpi = consts.tile([128, 1], F32)
        nc.vector.memset(hpi, float(np.pi / 2))

        # ---------------- MLP (feature-major, fp32) ----------------
        # PE fp32 here is ~1.4x a bf16 matmul per (k,m) pair, so plain fp32
        # beats hi/lo bf16 splitting (3-4 matmuls/pair) outright and is
        # exact w.r.t. the fp32 reference MLP.
        with tc.tile_pool(name="weights", bufs=1) as wpool:
            def stage(dram, shape, tag):
                t = wpool.tile(shape, F32, name=tag, tag=tag)
                nc.sync.dma_start(out=t, in_=dram[:, :, :])
                return t

            # critical path first: x0 + W1, then the rest streams behind
            x0 = stage(x0d, [128, 3, B_LOC], "x0")
            w1 = stage(w1d, [128, 3, H1], "w1")
            w2 = stage(w2d, [128, 8, H2], "w2")
            w3 = stage(w3d, [128, 8, H3], "w3")
            w4 = stage(w4d, [128, 4, DOUT], "w4")

            def dense(x, n_k, w, n_m, bias0, relu, out_t, m_widths=None):
                # out[feat, batch] = act(W.T @ x + b); K = n_k*128 partitions
                for mo in range(n_m):
                    if m_widths is None:
                        mw, m_lo = 128, mo * 128
                    else:
                        m_lo, mw = m_widths[mo]
                    ps = psmm.tile([128, B_LOC], F32, tag="mm")
                    for k in range(n_k):
                        nc.tensor.matmul(ps[0:mw, :],
                                         lhsT=w[:, k, m_lo:m_lo + mw],
                                         rhs=x[:, k, :],
                                         start=(k == 0), stop=(k == n_k - 1))
                    bcol = bs[0:mw, bias0 + mo:bias0 + mo + 1]
                    if relu:
                        nc.scalar.activation(
                            out=out_t[0:mw, mo, :], in_=ps[0:mw, :],
                            func=mybir.ActivationFunctionType.Relu,
                            bias=bcol, scale=1.0)
                    else:
                        nc.vector.tensor_scalar(
                            out=out_t[0:mw, mo, :], in0=ps[0:mw, :],
                            scalar1=bcol, scalar2=None,
                            op0=mybir.AluOpType.add)

            h1 = wpool.tile([128, 8, B_LOC], F32, name="h1", tag="h1")
            dense(x0, 3, w1, 8, 0, True, h1)
            h2 = wpool.tile([128, 8, B_LOC], F32, name="h2", tag="h2")
            dense(h1, 8, w2, 8, 8, True, h2)
            h3 = wpool.tile([128, 4, B_LOC], F32, name="h3", tag="h3")
            dense(h2, 8, w3, 4, 16, True, h3)
            # Layer 4 (no relu), fp32 out, aligned chunks:
            #   [0:100] Re(theta_raw), [100:200] Im(theta_raw), [200:264] F
            thp = acts.tile([128, 3, B_LOC], F32)
            dense(h3, 4, w4, 3, 20, False, thp,
                  m_widths=[(0, 100), (100, 100), (200, 64)])

        # ---------------- unit-modulus theta (feature-major) ----------------
        p_re = thp[0:100, 0, :]
        p_im = thp[0:100, 1, :]
        sq = tsc_pool.tile([100, B_LOC], F32, tag="sq")
        sq2 = tsc_pool.tile([100, B_LOC], F32, tag="sq2")
        nc.vector.tensor_mul(sq, p_re, p_re)
        nc.vector.tensor_mul(sq2, p_im, p_im)
        nc.vector.tensor_add(sq, sq, sq2)
        nc.scalar.sqrt(sq, sq)
        nc.vector.reciprocal(sq, sq)               # sq = 1/|theta|
        # [128, *] tiles so PE transposes use a full K=128 contraction
        # (sub-128-K matmuls misbehave); rows 100:128 are garbage that the
        # post-transpose column slice discards.
        a_fm = theta_pool.tile([128, B_LOC], F32)  # Re(theta), unit modulus
        b_fm = theta_pool.tile([128, B_LOC], F32)  # Im(theta)
        nc.vector.tensor_mul(a_fm[0:100, :], p_re, sq)
        nc.vector.tensor_mul(b_fm[0:100, :], p_im, sq)

        # ---------------- per-group sample-major processing ----------------
        # obuild opens after the weights pool closed: its arena reuses the
        # freed weight space so peak SBUF stays under the Tile cap.
        obuild_cm = tc.tile_pool(name="obuild", bufs=2)
        obuild = obuild_cm.__enter__()
        dpool_cm = tc.tile_pool(name="dpool", bufs=1)
        dpool = dpool_cm.__enter__()
        ccc_all = theta_pool.tile([128, N_GROUPS, C], F32)
        for g in range(N_GROUPS):
            gs = slice(g * 128, (g + 1) * 128)

            def to_sample_major(src_fm, np_, tag):
                # src [128, 128] feature-major slice -> [128, np_] sample-major
                ps = pstr.tile([128, 128], F32, tag="tr")
                nc.tensor.transpose(ps, src_fm, ident)
                dst = theta_pool.tile([128, np_], F32, name=tag, tag=tag)
                nc.scalar.copy(out=dst, in_=ps[:, 0:np_])
                return dst

            a_pack = to_sample_major(a_fm[:, gs], 100, f"apack{g}")
            b_pack = to_sample_major(b_fm[:, gs], 100, f"bpack{g}")
            f_pack = to_sample_major(thp[:, 2, gs], 64, f"fpack{g}")

            # ---- F1/F2 precoder normalization (sample-major) ----
            fsq = tsc_pool.tile([128, 2, 32], F32, tag="fsq")
            f_v = f_pack[:].rearrange("p (g2 i) -> p g2 i", g2=2)
            nc.vector.tensor_mul(fsq, f_v, f_v)
            fnorm = tsc_pool.tile([128, 2], F32, tag="fnorm")
            nc.vector.reduce_sum(fnorm, fsq, axis=mybir.AxisListType.X)
            # scale = sqrt(2/norm) = 1/sqrt(norm*0.5)
            nc.scalar.activation(out=fnorm, in_=fnorm,
                                 func=mybir.ActivationFunctionType.Sqrt, scale=0.5)
            nc.vector.reciprocal(fnorm, fnorm)
            fhat = theta_pool.tile([128, 2, 32], F32, name=f"fhat{g}", tag=f"fhat{g}")
            nc.vector.tensor_mul(fhat, f_v,
                                 fnorm[:, :, None].to_broadcast((128, 2, 32)))
            nc.sync.dma_start(out=out[gs, 200:264],
                              in_=fhat[:].rearrange("p g2 i -> p (g2 i)"))

            # ---- packed fp32 outer products over circular diagonals ----
            # theta is unit-modulus, so with psi = atan2(b, a) / 2:
            #   ovec[:,0,k,n] = a_n a_m + b_n b_m = cos(d) = 1 - 2 sin(dp)^2
            #   ovec[:,1,k,n] = b_n a_m - a_n b_m = sin(d) = 2 sin(dp) cos(dp)
            # where d = phi_m - phi_n, dp = d/2 in [-pi, pi], m = (n+k)%100.
            # The HW Sin table is fp32-accurate only on ~[-3.2, 3.2], so all
            # arguments are kept in [-pi, pi] (cos via Sin(pi/2 - |dp|)).
            # This needs exactly ONE overlapping-stride op per group (the
            # dp diagonal subtract, on the otherwise idle GPSIMD) instead
            # of four outer-product multiplies.
            # psi = atan2(b, a) / 2 via octant-folded arctan (argument kept
            # in [0, 1] -- both the HW table and CoreSim accept that range)
            ts_ = nc.vector.tensor_scalar
            tt_ = nc.vector.tensor_tensor
            def small(tag):
                return tsc_pool.tile([128, NRIS], F32, name=tag, tag=tag)
            absa, absb = small("absa"), small("absb")
            nc.scalar.activation(out=absa, in_=a_pack,
                                 func=mybir.ActivationFunctionType.Abs)
            nc.scalar.activation(out=absb, in_=b_pack,
                                 func=mybir.ActivationFunctionType.Abs)
            lo, hi = small("lo"), small("hi")
            tt_(lo, absa, absb, mybir.AluOpType.min)
            tt_(hi, absa, absb, mybir.AluOpType.max)
            nc.vector.reciprocal(hi, hi)
            tt_(lo, lo, hi, mybir.AluOpType.mult)        # r in [0, 1]
            base = small("base")
            nc.scalar.activation(out=base, in_=lo,
                                 func=mybir.ActivationFunctionType.Arctan)
            # swap = (|b| > |a|) as 0/1; at |a| == |b| exactly the blend
            # coefficient multiplies zero, so sign(0)=0 -> 0.5 is harmless
            swap = small("swap")
            tt_(swap, absb, absa, mybir.AluOpType.subtract)
            nc.scalar.activation(out=swap, in_=swap,
                                 func=mybir.ActivationFunctionType.Sign)
            ts_(out=swap, in0=swap, scalar1=0.5, scalar2=0.5,
                op0=mybir.AluOpType.mult, op1=mybir.AluOpType.add)
            # v = atan2(|b|, |a|) = base + swap*(pi/2 - 2*base)
            t2 = small("t2")
            ts_(out=t2, in0=base, scalar1=-2.0, scalar2=float(np.pi / 2),
                op0=mybir.AluOpType.mult, op1=mybir.AluOpType.add)
            tt_(t2, t2, swap, mybir.AluOpType.mult)
            tt_(base, base, t2, mybir.AluOpType.add)     # v
            s_a, s_b = small("sa"), small("sb")
            nc.scalar.activation(out=s_a, in_=a_pack,
                                 func=mybir.ActivationFunctionType.Sign)
            nc.scalar.activation(out=s_b, in_=b_pack,
                                 func=mybir.ActivationFunctionType.Sign)
            # phi = sign(b) * (v + (a<0)*(pi - 2v));  psi = phi/2
            ts_(out=t2, in0=base, scalar1=-2.0, scalar2=float(np.pi),
                op0=mybir.AluOpType.mult, op1=mybir.AluOpType.add)
            ts_(out=s_a, in0=s_a, scalar1=-0.5, scalar2=0.5,
                op0=mybir.AluOpType.mult, op1=mybir.AluOpType.add)
            tt_(t2, t2, s_a, mybir.AluOpType.mult)
            tt_(base, base, t2, mybir.AluOpType.add)
            ts_(out=s_b, in0=s_b, scalar1=0.5, scalar2=None,
                op0=mybir.AluOpType.mult)
            psi = theta_pool.tile([128, NRIS], F32, name=f"psi{g}", tag=f"psi{g}")
            tt_(psi, base, s_b, mybir.AluOpType.mult)
            psi_ext = theta_pool.tile([128, 2 * NRIS], F32,
                                      name=f"psiext{g}", tag=f"psiext{g}")
            nc.scalar.copy(out=psi_ext[:, 0:NRIS], in_=psi)
            nc.scalar.copy(out=psi_ext[:, NRIS:2 * NRIS], in_=psi)
            sh3 = (128, NK, NRIS)
            dpsi = dpool.tile([128, NK, NRIS], F32, tag="dpsi")
            stile = dpool.tile([128, NK, NRIS], F32, tag="stile")
            # dp[k,n] = psi[(n+k)%100] - psi[n]  (GPSIMD, diag view)
            nc.gpsimd.tensor_tensor(dpsi, _diag_view(psi_ext, sh3),
                                    psi[:, None, :].to_broadcast(sh3),
                                    mybir.AluOpType.subtract)
            ovec = obuild.tile([128, 2, NK, NRIS], F32, tag="ovec")
            # s = sin(dp) on ACT
            nc.scalar.activation(out=stile, in_=dpsi,
                                 func=mybir.ActivationFunctionType.Sin)
            # o1 = 1 - 2 s^2:  GPSIMD squares, DVE affine-finishes in place
            nc.gpsimd.tensor_tensor(ovec[:, 0], stile, stile,
                                    mybir.AluOpType.mult)
            nc.vector.tensor_scalar(out=ovec[:, 0], in0=ovec[:, 0],
                                    scalar1=-2.0, scalar2=1.0,
                                    op0=mybir.AluOpType.mult,
                                    op1=mybir.AluOpType.add)
            # c = cos(dp) = Sin(pi/2 - |dp|), computed in place over dp
            nc.scalar.activation(out=dpsi, in_=dpsi,
                                 func=mybir.ActivationFunctionType.Abs)
            nc.scalar.activation(out=dpsi, in_=dpsi,
                                 func=mybir.ActivationFunctionType.Sin,
                                 bias=hpi[:, 0:1], scale=-1.0)
            # o2 = b_n a_m - a_n b_m = sin(phi_n - phi_m) = -2 s c on DVE
            nc.vector.scalar_tensor_tensor(out=ovec[:, 1], in0=stile,
                                           scalar=-2.0, in1=dpsi,
                                           op0=mybir.AluOpType.mult,
                                           op1=mybir.AluOpType.mult)

            # ---- stream packed T; per chunk: DVE multiply, ACT reduce ----
            # (tensor_tensor_reduce crashes this HW stack, so the reduce
            # rides scalar-engine activation(Copy, accum_out=...); the 1e15
            # CCC scale is applied later in fp32)
            parts = theta_pool.tile([128, C, 2], F32,
                                    name=f"parts{g}", tag=f"parts{g}")
            ccc = ccc_all[:, g, :]
            for side in range(2):
                for c in range(C):
                    chunk = tch_pool.tile([128, NK, NRIS], F32, tag="tchunk")
                    src = tpack[gs, c, side]
                    if T_DMA == "swdge":
                        nc.gpsimd.dma_start(out=chunk, in_=src)
                    else:
                        nc.sync.dma_start(out=chunk, in_=src)
                    nc.vector.tensor_mul(chunk, chunk, ovec[:, side])
                    nc.scalar.activation(
                        out=chunk, in_=chunk,
                        func=mybir.ActivationFunctionType.Copy,
                        bias=0.0, scale=1.0,
                        accum_out=parts[:, c, side:side + 1])
            nc.vector.reduce_sum(ccc, parts, axis=mybir.AxisListType.X)

            # ---- scale = rsqrt(max(CCC*1e15, 1)) ; theta_hat ----
            mx = tsc_pool.tile([128, 1], F32, tag="mx")
            nc.vector.reduce_max(mx, ccc, axis=mybir.AxisListType.X)
            nc.vector.tensor_scalar(out=mx, in0=mx, scalar1=INV_THRESH,
                                    scalar2=1.0, op0=mybir.AluOpType.mult,
                                    op1=mybir.AluOpType.max)
            nc.scalar.sqrt(mx, mx)
            nc.vector.reciprocal(mx, mx)
            th_re = theta_pool.tile([128, NRIS], F32, name=f"thre{g}", tag=f"thre{g}")
            th_im = theta_pool.tile([128, NRIS], F32, name=f"thim{g}", tag=f"thim{g}")
            nc.vector.tensor_scalar_mul(th_re, a_pack, mx)
            nc.vector.tensor_scalar_mul(th_im, b_pack, mx)
            nc.sync.dma_start(out=out[gs, 0:100], in_=th_re)
            nc.sync.dma_start(out=out[gs, 100:200], in_=th_im)
        dpool_cm.__exit__(None, None, None)
        obuild_cm.__exit__(None, None, None)


_NC_LOCK = threading.Lock()
_NC = None


def _get_nc():
    global _NC
    with _NC_LOCK:
        if _NC is None:
            _NC = build_nc()
    return _NC


def _wprep(W, n_k):
    # [K, M] fp32 -> lhsT layout [p, o, m] with k = o*128 + p
    K, M = W.shape
    Wp = np.zeros((n_k * 128, M), np.float32)
    Wp[:K] = W
    return np.ascontiguousarray(Wp.reshape(n_k, 128, M).transpose(1, 0, 2))


def _pack_T(t_re, t_im):
    """[B, C, 100, 100] fp32 pair -> [B, C, 2, NK, NRIS] circular-diagonal
    packed Ts/Ta (see module docstring)."""
    k_ix = np.arange(NK)[:, None]
    n_ix = np.arange(NRIS)[None, :]
    m_ix = (n_ix + k_ix) % NRIS                      # [NK, 100]
    n_bx = np.broadcast_to(n_ix, (NK, NRIS))
    ts = t_re[:, :, n_bx, m_ix] + t_re[:, :, m_ix, n_bx]
    ta = t_im[:, :, n_bx, m_ix] - t_im[:, :, m_ix, n_bx]
    ts[:, :, 0, :] = t_re.diagonal(axis1=2, axis2=3)  # k=0: plain diag
    ta[:, :, 0, :] = 0.0
    ts[:, :, 50, 50:] = 0.0                           # k=50: half, no double count
    ta[:, :, 50, 50:] = 0.0
    return np.ascontiguousarray(
        np.stack([ts, ta], axis=2).astype(np.float32))


def _prep_shared(inputs):
    """Host-side prep of replicated tensors (weights/biases) + packed T."""
    w1 = _wprep(np.asarray(inputs["W1"], np.float32), 3)
    w2 = _wprep(np.asarray(inputs["W2"], np.float32), 8)
    w3 = _wprep(np.asarray(inputs["W3"], np.float32), 8)
    w4 = _wprep(np.asarray(inputs["W4"], np.float32), 4)
    biases = np.zeros((128, 24), np.float32)
    biases[:, 0:8] = np.asarray(inputs["b1"], np.float32).reshape(8, 128).T
    biases[:, 8:16] = np.asarray(inputs["b2"], np.float32).reshape(8, 128).T
    biases[:, 16:20] = np.asarray(inputs["b3"], np.float32).reshape(4, 128).T
    b4 = np.asarray(inputs["b4"], np.float32)
    biases[0:100, 20] = b4[0:100]
    biases[0:100, 21] = b4[100:200]
    biases[0:64, 22] = b4[200:264]
    tpack = _pack_T(np.asarray(inputs["T_real"], np.float32),
                    np.asarray(inputs["T_imag"], np.float32))
    return w1, w2, w3, w4, biases, tpack


def _shard_inputs(inputs):
    w1, w2, w3, w4, biases, tpack = _prep_shared(inputs)
    s1 = np.asarray(inputs["sample1"], np.float32)
    in_maps = []
    for i in range(N_CORES):
        bsl = slice(i * B_LOC, (i + 1) * B_LOC)
        x = np.zeros((DIN_PAD, B_LOC), np.float32)
        x[:DIN] = s1[bsl].T
        x0 = np.ascontiguousarray(x.reshape(3, 128, B_LOC).transpose(1, 0, 2))
        in_maps.append({
            "x0": x0, "w1": w1, "w2": w2, "w3": w3, "w4": w4,
            "biases": biases,
            "tpack": tpack[bsl],
        })
    return in_maps


def run_on_hw(inputs, trace=False, **kwargs):
    nc = _get_nc()
    res = run_bass_kernel_spmd(nc, _shard_inputs(inputs),
                               list(range(N_CORES)), trace=trace, **kwargs)
    full = np.concatenate([res.results[i]["out"] for i in range(N_CORES)], axis=0)
    return full, res


def kernel(**inputs) -> np.ndarray:
    full, _ = run_on_hw(inputs, trace=False)
    return full.astype(np.float32)
